# revision 23
# baseline (speedup 1.0000x reference)
"""Trainium2 Bass kernel for nn_MultiHeadAttention_84576495993495.

Key observation: the reference module's output einsum is
    out = einsum('bhqk,bhvo->bhvo', attn, v)
which contracts softmax(attn) over BOTH q and k. Every softmax row sums
to 1, so sum_{q,k} attn == S (= 2048) and the whole attention block
collapses to out == S * v. Hence

    reference(x, ...) == ((x @ Wv.T + bv) * S) @ Wp.T + bp
                      ==  x @ M + c
with
    M = S * Wv.T @ Wp.T          (folded on host in fp64, then split)
    c = S * Wp @ bv + bp

(Verified vs the jax reference: rel Frobenius err ~3.6e-7 = fp32 noise.)

Device work: the data-dependent GEMM y = x @ M + c, sharded
data-parallel over the 8192 rows -> 1024 rows per NeuronCore.

Precision strategy: the harness tolerance is 2e-2; a SINGLE fp16 pass
(x, M both rounded to fp16, fp32 PSUM accumulate) gives rel err 2.5e-4
-- 80x inside tolerance -- at 1/3 the PE time of the fp16x3 split
scheme (which reaches 2.5e-7 but this problem does not need it).
The output is stored fp16 (adds ~2.8e-4 quantization, still ~50x
inside tolerance) which halves the output DMA traffic and doubles the
DVE tail write rate; the host upcasts to fp32.

The GEMM runs in bf16 (rel err 2.0e-3 on HW): measured ~4% faster than
fp16 at 8 concurrent cores (3.2e-4), both far inside tolerance.

Schedule (per 512-col n-chunk): k-major accumulation staggered over
two groups of 4 PSUM banks, so each group's 32 matmuls overlap the
OTHER group's tails (bias add on DVE + store). Tails alternate between
the SP and Activation HWDGE rings so output DMA never serializes
behind one descriptor queue. Microbenchmarked on HW (For_i steady
state): rgroup order with tails+DMA = 35.4us vs 43.1us for flat
k-major (1 core); pure GEMM floor is 34.7us at 8 cores (bf16 streams
at ~0.53-0.60 ns/row sustained for every operand pattern tried -- the
cost model's 2.4 GHz / 0.417 ns/row is never reached; fp8 only buys
~1.2x per row, so split-fp8 schemes lose to one bf16 pass). Groups of
2 banks, ap=256 accumulators (PSUM pool is bank-granular), fp32
output, contiguous tiled-y stores, and copy-tails-with-host-bias were
all tried on HW and did not beat this configuration.

Measured on HW (8 cores, axon, steady-state For_i slope over
T in {1, 8193, 16385} -- same protocol as the 137667ns baseline):
~43-46us, i.e. ~3x the baseline. See test.py output.
"""

import os
from functools import lru_cache

import numpy as np

# Defensive: a previous run crashing mid-execution can leave the NeuronCores
# in an unrecoverable state (NRT_EXEC_UNIT_UNRECOVERABLE); resetting cores at
# NRT init clears it and is harmless otherwise.
os.environ.setdefault("NEURON_RT_RESET_CORES", "1")

import concourse.bass as bass
import concourse.mybir as mybir
import concourse.tile as tile
from concourse import bacc
from concourse.bass_utils import run_bass_kernel_spmd

N_CORES = 8
P = 128
D = 1024                       # model dim (= SLICE_SIZE)
B, S = 4, 2048
R_TOTAL = B * S                # 8192 rows
R_CORE = R_TOTAL // N_CORES    # 1024 rows per core
K_TILES = D // P               # 8
R_TILES = R_CORE // P          # 8
N_CHUNK = int(os.environ.get("KMM_NCHUNK", "512"))  # PSUM cols per group
N_CHUNKS = D // N_CHUNK
PS_BUFS = (8 * 2048) // (N_CHUNK * 4)  # PSUM accumulators that fit
R_GROUP = int(os.environ.get("KMM_RGROUP", "4"))  # banks per stagger group
SCALE = float(S)               # sum over q,k of softmax rows == S

# "bf16x1" (default) | "fp16x1" | "fp16x3" | "float32" | "float32r"
MM_MODE = os.environ.get("KMM_DTYPE", "bf16x1")
# output dtype: "f16" (default; host upcasts) | "f32"
OUT_MODE = os.environ.get("KMM_OUT", "f16")
# tail: "add" (DVE adds bias, device-complete) | "copy" (DVE copy, bias
# added on host during upcast) | "psum" (DMA directly from PSUM, bias on
# host; no DVE in the chain)
TAIL_MODE = os.environ.get("KMM_TAIL", "add")
# y layout: "rowmajor" | "tiled" ([r, nch, 128, 512] so each store is one
# fully contiguous DRAM block; host reassembles)
Y_LAYOUT = os.environ.get("KMM_YLAYOUT", "rowmajor")
OPOOL_BUFS = int(os.environ.get("KMM_OPOOL", "16"))
# tail engine: "vector" | "split" (alternate DVE / GpSimd)
TAIL_ENG = os.environ.get("KMM_TAILENG", "vector")


def _mm_dt(mode):
    if mode in ("fp16x3", "fp16x1"):
        return mybir.dt.float16
    if mode == "bf16x1":
        return mybir.dt.bfloat16
    return getattr(mybir.dt, mode)


@lru_cache(maxsize=4)
def _build_nc(mode: str, loop_iters: int | None = None, order: str | None = None):
    """loop_iters: when set, wrap the compute body in a tc.For_i hardware
    loop (inputs loaded once) -- used by the benchmark harness to measure
    steady-state per-iteration device time without NTFF profiling."""
    if order is None:
        order = os.environ.get("KMM_ORDER", "rgroup")
    split = mode == "fp16x3"
    mm_dt = _mm_dt(mode)
    out_dt = mybir.dt.float16 if OUT_MODE == "f16" else mybir.dt.float32
    if TAIL_MODE == "psum":
        out_dt = mybir.dt.float32  # DMA cannot convert dtypes on HWDGE
    nc = bacc.Bacc(None, target_bir_lowering=False)

    if split:
        x_names, m_names = ["xh", "xl"], ["Mh", "Ml"]
    else:
        x_names, m_names = ["xh"], ["Mh"]
    x_dram = [
        nc.dram_tensor(n, [D, R_CORE], mm_dt, kind="ExternalInput") for n in x_names
    ]
    m_dram = [nc.dram_tensor(n, [D, D], mm_dt, kind="ExternalInput") for n in m_names]
    cb = nc.dram_tensor("cb", [P, D], mybir.dt.float32, kind="ExternalInput")
    if Y_LAYOUT == "tiled":
        y = nc.dram_tensor(
            "y", [R_TILES, N_CHUNKS, P, N_CHUNK], out_dt, kind="ExternalOutput"
        )
    else:
        y = nc.dram_tensor("y", [R_CORE, D], out_dt, kind="ExternalOutput")

    x_t = [t.rearrange("(ko p) r -> p ko r", p=P) for t in x_dram]   # [128, 8, 1024]
    m_t = [t.rearrange("(ko p) n -> p ko n", p=P) for t in m_dram]   # [128, 8, 1024]

    # (x operand, M operand) per accumulation pass; the xl@Ml term is dropped.
    passes = [(0, 0), (0, 1), (1, 0)] if split else [(0, 0)]

    with tile.TileContext(nc) as tc:
        with (
            tc.tile_pool(name="wpool", bufs=1) as wpool,
            tc.tile_pool(name="opool", bufs=OPOOL_BUFS) as opool,
            tc.tile_pool(name="pspool", bufs=PS_BUFS, space="PSUM") as pspool,
        ):
            x_sb = [
                wpool.tile([P, K_TILES, R_CORE], mm_dt, tag=f"x_sb{i}", name=f"x_sb{i}")
                for i in range(len(x_dram))
            ]
            m_sb = [
                wpool.tile([P, K_TILES, D], mm_dt, tag=f"m_sb{i}", name=f"m_sb{i}")
                for i in range(len(m_dram))
            ]
            cb_sb = wpool.tile([P, D], mybir.dt.float32, tag="cb_sb")

            if TAIL_MODE == "add":
                nc.sync.dma_start(cb_sb[:], cb[:])
            # Load in consumption order, alternating the two HWDGE rings.
            for i in range(len(x_dram)):
                for k in range(K_TILES):
                    nc.sync.dma_start(x_sb[i][:, k], x_t[i][:, k])
                    for nch in range(N_CHUNKS):
                        nc.scalar.dma_start(
                            m_sb[i][:, k, bass.ts(nch, N_CHUNK)],
                            m_t[i][:, k, bass.ts(nch, N_CHUNK)],
                        )

            n_acc = len(passes) * K_TILES

            def y_dst(r, nch):
                if Y_LAYOUT == "tiled":
                    return y[r, nch]
                return y[bass.ts(r, P), bass.ts(nch, N_CHUNK)]

            def emit_tail(r, nch, ps):
                eng = nc.sync if r % 2 == 0 else nc.scalar
                if TAIL_MODE == "psum":
                    eng.dma_start(y_dst(r, nch), ps[:])
                    return
                out_sb = opool.tile([P, N_CHUNK], out_dt, tag="out_sb")
                veng = (
                    nc.vector
                    if (TAIL_ENG == "vector" or r % 2 == 0)
                    else nc.gpsimd
                )
                if TAIL_MODE == "copy":
                    veng.tensor_copy(out_sb[:], ps[:])
                else:
                    veng.tensor_add(
                        out_sb[:], ps[:], cb_sb[:, bass.ts(nch, N_CHUNK)]
                    )
                eng.dma_start(y_dst(r, nch), out_sb[:])

            def body_rgroup():
                # k-major within staggered groups of R_GROUP banks: each
                # group's tails overlap the next group's matmuls.
                for nch in range(N_CHUNKS):
                    groups = [
                        pspool.tile([P, N_CHUNK], mybir.dt.float32, tag="ps", name="ps")
                        for _ in range(R_TILES)
                    ]
                    for g0 in range(0, R_TILES, R_GROUP):
                        step = 0
                        for xi, mi in passes:
                            for k in range(K_TILES):
                                for r in range(g0, g0 + R_GROUP):
                                    nc.tensor.matmul(
                                        groups[r][:],
                                        x_sb[xi][:, k, bass.ts(r, P)],
                                        m_sb[mi][:, k, bass.ts(nch, N_CHUNK)],
                                        start=(step == 0),
                                        stop=(step == n_acc - 1),
                                    )
                                step += 1
                        for r in range(g0, g0 + R_GROUP):
                            emit_tail(r, nch, groups[r])

            def body_kmajor():
                # k-major across 8 live PSUM banks (bank switch every MM)
                for nch in range(N_CHUNKS):
                    groups = [
                        pspool.tile([P, N_CHUNK], mybir.dt.float32, tag="ps", name="ps")
                        for _ in range(R_TILES)
                    ]
                    step = 0
                    for xi, mi in passes:
                        for k in range(K_TILES):
                            for r in range(R_TILES):
                                nc.tensor.matmul(
                                    groups[r][:],
                                    x_sb[xi][:, k, bass.ts(r, P)],
                                    m_sb[mi][:, k, bass.ts(nch, N_CHUNK)],
                                    start=(step == 0),
                                    stop=(step == n_acc - 1),
                                )
                            step += 1
                    for r in range(R_TILES):
                        emit_tail(r, nch, groups[r])

            body = body_rgroup if order == "rgroup" else body_kmajor

            if loop_iters is None:
                body()
            else:
                with tc.For_i(0, loop_iters, 1):
                    body()
    nc.compile()
    return nc


def _np_dt(mode):
    if mode in ("fp16x3", "fp16x1"):
        return np.float16
    if mode == "bf16x1":
        import ml_dtypes

        return ml_dtypes.bfloat16
    return np.float32


def _host_prep(x, Wv, bv, Wp, bp, mode=None):
    mode = mode or MM_MODE
    np_dt = _np_dt(mode)
    X = np.ascontiguousarray(x, dtype=np.float32).reshape(R_TOTAL, D)
    M64 = SCALE * (Wv.T.astype(np.float64) @ Wp.T.astype(np.float64))
    c = (SCALE * (Wp.astype(np.float64) @ bv.astype(np.float64)) + bp).astype(
        np.float32
    )
    cbt = np.ascontiguousarray(np.broadcast_to(c, (P, D)))

    if mode == "fp16x3":
        Mh = M64.astype(np.float16)
        Ml = (M64 - Mh.astype(np.float64)).astype(np.float16)
        m_arrs = {"Mh": Mh, "Ml": Ml}
    else:
        m_arrs = {"Mh": M64.astype(np_dt)}

    in_maps = []
    for i in range(N_CORES):
        shard_t = np.ascontiguousarray(X[i * R_CORE : (i + 1) * R_CORE].T)
        im = dict(m_arrs)
        im["cb"] = cbt
        if mode == "fp16x3":
            xh = shard_t.astype(np.float16)
            xl = (shard_t - xh.astype(np.float32)).astype(np.float16)
            im["xh"] = xh
            im["xl"] = xl
        else:
            im["xh"] = shard_t.astype(np_dt)
        in_maps.append(im)
    return in_maps


def kernel(x, Wq, bq, Wk, bk, Wv, bv, Wp, bp):
    x, Wv, bv, Wp, bp = (np.asarray(a) for a in (x, Wv, bv, Wp, bp))
    nc = _build_nc(MM_MODE)
    in_maps = _host_prep(x, Wv, bv, Wp, bp)
    res = run_bass_kernel_spmd(nc, in_maps, core_ids=list(range(N_CORES)))

    def fix(a):
        a = np.asarray(a)
        if Y_LAYOUT == "tiled":  # [r, nch, 128, 512] -> [1024, 1024]
            a = a.transpose(0, 2, 1, 3).reshape(R_CORE, D)
        return a.astype(np.float32)

    y = np.concatenate([fix(r["y"]) for r in res.results], axis=0)
    if TAIL_MODE != "add":  # bias was not applied on device
        c = (
            SCALE * (Wp.astype(np.float64) @ bv.astype(np.float64)) + bp
        ).astype(np.float32)
        y += c
    return y.reshape(B, S, D)


# revision 35
# speedup vs baseline: 1.0636x; 1.0636x over previous
"""Trainium2 Bass kernel for nn_MultiHeadAttention_84576495993495.

Key observation: the reference module's output einsum is
    out = einsum('bhqk,bhvo->bhvo', attn, v)
which contracts softmax(attn) over BOTH q and k. Every softmax row sums
to 1, so sum_{q,k} attn == S (= 2048) and the whole attention block
collapses to out == S * v. Hence

    reference(x, ...) == ((x @ Wv.T + bv) * S) @ Wp.T + bp
                      ==  x @ M + c
with
    M = S * Wv.T @ Wp.T          (folded on host in fp64, then split)
    c = S * Wp @ bv + bp

(Verified vs the jax reference: rel Frobenius err ~3.6e-7 = fp32 noise.)

Device work: the data-dependent GEMM y = x @ M + c, sharded
data-parallel over the 8192 rows -> 1024 rows per NeuronCore.

Precision strategy: the harness tolerance is 2e-2; a SINGLE fp16 pass
(x, M both rounded to fp16, fp32 PSUM accumulate) gives rel err 2.5e-4
-- 80x inside tolerance -- at 1/3 the PE time of the fp16x3 split
scheme (which reaches 2.5e-7 but this problem does not need it).
The output is stored fp16 (adds ~2.8e-4 quantization, still ~50x
inside tolerance) which halves the output DMA traffic and doubles the
DVE tail write rate; the host upcasts to fp32.

The GEMM runs in bf16 (rel err 2.0e-3 on HW): measured ~4% faster than
fp16 at 8 concurrent cores (3.2e-4), both far inside tolerance.

Schedule (per 512-col n-chunk): k-major accumulation staggered over
two groups of 4 PSUM banks, so each group's 32 matmuls overlap the
OTHER group's tails (bias add on DVE + store). Tails alternate between
the SP and Activation HWDGE rings so output DMA never serializes
behind one descriptor queue. Microbenchmarked on HW (For_i steady
state): rgroup order with tails+DMA = 35.4us vs 43.1us for flat
k-major (1 core); pure GEMM floor is 34.7us at 8 cores (bf16 streams
at ~0.53-0.60 ns/row sustained for every operand pattern tried -- the
cost model's 2.4 GHz / 0.417 ns/row is never reached; fp8 only buys
~1.2x per row, so split-fp8 schemes lose to one bf16 pass). Groups of
2 banks, ap=256 accumulators (PSUM pool is bank-granular), fp32
output, contiguous tiled-y stores, and copy-tails-with-host-bias were
all tried on HW and did not beat this configuration.

Measured on HW (8 cores, axon, steady-state For_i slope over
T in {1, 8193, 16385} -- same protocol as the 137667ns baseline):
35.9-48.4us across repeated sessions of this exact config, rel err
2.008e-03; ~2.8-3.8x the baseline. In-session attribution at 8 cores:
GEMM only 32.3us, +DVE tails 36.3us, +stores 46.3us -- the stores are
the main non-GEMM cost and behave descriptor-rate-limited (~40M
descriptors/s/core; fp32 stores move 2x the bytes in the same time).
Merged-row stores (half the descriptors) and a whole-iteration
partition-major store (1/16th) were built and A/B'd order-controlled:
within slot-drift noise of "per" (sessions drift ~1.5us per
successive config and +/-5us session-to-session, so late-running
configs measure slow). Also tried without improvement: a third store
ring via gpsimd SWDGE, opool=32, gpsimd tails (cannot read PSUM),
DMA-from-PSUM (asserted unsupported), contiguous tiled-y stores.
"""

import os
from functools import lru_cache

import numpy as np

# Defensive: a previous run crashing mid-execution can leave the NeuronCores
# in an unrecoverable state (NRT_EXEC_UNIT_UNRECOVERABLE); resetting cores at
# NRT init clears it and is harmless otherwise.
os.environ.setdefault("NEURON_RT_RESET_CORES", "1")

import concourse.bass as bass
import concourse.mybir as mybir
import concourse.tile as tile
from concourse import bacc
from concourse.bass_utils import run_bass_kernel_spmd

N_CORES = 8
P = 128
D = 1024                       # model dim (= SLICE_SIZE)
B, S = 4, 2048
R_TOTAL = B * S                # 8192 rows
R_CORE = R_TOTAL // N_CORES    # 1024 rows per core
K_TILES = D // P               # 8
R_TILES = R_CORE // P          # 8
N_CHUNK = int(os.environ.get("KMM_NCHUNK", "512"))  # PSUM cols per group
N_CHUNKS = D // N_CHUNK
PS_BUFS = (8 * 2048) // (N_CHUNK * 4)  # PSUM accumulators that fit
R_GROUP = int(os.environ.get("KMM_RGROUP", "4"))  # banks per stagger group
SCALE = float(S)               # sum over q,k of softmax rows == S

# "bf16x1" (default) | "fp16x1" | "fp16x3" | "float32" | "float32r"
MM_MODE = os.environ.get("KMM_DTYPE", "bf16x1")
# output dtype: "f16" (default; host upcasts) | "f32"
OUT_MODE = os.environ.get("KMM_OUT", "f16")
# tail: "add" (DVE adds bias, device-complete) | "copy" (DVE copy, bias
# added on host during upcast) | "psum" (DMA directly from PSUM, bias on
# host; no DVE in the chain)
TAIL_MODE = os.environ.get("KMM_TAIL", "add")
# y layout: "rowmajor" | "tiled" ([r, nch, 128, 512] so each store is one
# fully contiguous DRAM block; host reassembles)
Y_LAYOUT = os.environ.get("KMM_YLAYOUT", "rowmajor")
OPOOL_BUFS = int(os.environ.get("KMM_OPOOL", "16"))
# tail engine: "vector" | "split" (alternate DVE / GpSimd)
TAIL_ENG = os.environ.get("KMM_TAILENG", "vector")
# store granularity: "per" (one store per bank+nchunk, 1KB DRAM runs) |
# "merged" (both nchunks collected in one [P, D] tile per bank, stored as
# full 2KB rows -> half the DMA descriptors) | "mega" (whole iteration in
# one [P, R_TILES, D] tile, y kept partition-major in DRAM and reassembled
# on host -> 8KB runs, ~128 descriptors/iter; stores are descriptor-rate
# limited at 8 cores)
STORE_MODE = os.environ.get("KMM_STORE", "per")


def _mm_dt(mode):
    if mode in ("fp16x3", "fp16x1"):
        return mybir.dt.float16
    if mode == "bf16x1":
        return mybir.dt.bfloat16
    return getattr(mybir.dt, mode)


@lru_cache(maxsize=4)
def _build_nc(mode: str, loop_iters: int | None = None, order: str | None = None):
    """loop_iters: when set, wrap the compute body in a tc.For_i hardware
    loop (inputs loaded once) -- used by the benchmark harness to measure
    steady-state per-iteration device time without NTFF profiling."""
    if order is None:
        order = os.environ.get("KMM_ORDER", "rgroup")
    split = mode == "fp16x3"
    mm_dt = _mm_dt(mode)
    out_dt = mybir.dt.float16 if OUT_MODE == "f16" else mybir.dt.float32
    if TAIL_MODE == "psum":
        out_dt = mybir.dt.float32  # DMA cannot convert dtypes on HWDGE
    nc = bacc.Bacc(None, target_bir_lowering=False)

    if split:
        x_names, m_names = ["xh", "xl"], ["Mh", "Ml"]
    else:
        x_names, m_names = ["xh"], ["Mh"]
    x_dram = [
        nc.dram_tensor(n, [D, R_CORE], mm_dt, kind="ExternalInput") for n in x_names
    ]
    m_dram = [nc.dram_tensor(n, [D, D], mm_dt, kind="ExternalInput") for n in m_names]
    cb = nc.dram_tensor("cb", [P, D], mybir.dt.float32, kind="ExternalInput")
    if STORE_MODE == "mega":
        y = nc.dram_tensor(
            "y", [P, R_TILES, D], out_dt, kind="ExternalOutput"
        )
    elif Y_LAYOUT == "tiled":
        y = nc.dram_tensor(
            "y", [R_TILES, N_CHUNKS, P, N_CHUNK], out_dt, kind="ExternalOutput"
        )
    else:
        y = nc.dram_tensor("y", [R_CORE, D], out_dt, kind="ExternalOutput")

    x_t = [t.rearrange("(ko p) r -> p ko r", p=P) for t in x_dram]   # [128, 8, 1024]
    m_t = [t.rearrange("(ko p) n -> p ko n", p=P) for t in m_dram]   # [128, 8, 1024]

    # (x operand, M operand) per accumulation pass; the xl@Ml term is dropped.
    passes = [(0, 0), (0, 1), (1, 0)] if split else [(0, 0)]

    with tile.TileContext(nc) as tc:
        with (
            tc.tile_pool(name="wpool", bufs=1) as wpool,
            tc.tile_pool(
                name="opool",
                bufs=2 if STORE_MODE == "mega" else OPOOL_BUFS,
            ) as opool,
            tc.tile_pool(name="pspool", bufs=PS_BUFS, space="PSUM") as pspool,
        ):
            x_sb = [
                wpool.tile([P, K_TILES, R_CORE], mm_dt, tag=f"x_sb{i}", name=f"x_sb{i}")
                for i in range(len(x_dram))
            ]
            m_sb = [
                wpool.tile([P, K_TILES, D], mm_dt, tag=f"m_sb{i}", name=f"m_sb{i}")
                for i in range(len(m_dram))
            ]
            cb_sb = wpool.tile([P, D], mybir.dt.float32, tag="cb_sb")

            if TAIL_MODE == "add":
                nc.sync.dma_start(cb_sb[:], cb[:])
            # Load in consumption order, alternating the two HWDGE rings.
            for i in range(len(x_dram)):
                for k in range(K_TILES):
                    nc.sync.dma_start(x_sb[i][:, k], x_t[i][:, k])
                    for nch in range(N_CHUNKS):
                        nc.scalar.dma_start(
                            m_sb[i][:, k, bass.ts(nch, N_CHUNK)],
                            m_t[i][:, k, bass.ts(nch, N_CHUNK)],
                        )

            n_acc = len(passes) * K_TILES

            def y_dst(r, nch):
                if Y_LAYOUT == "tiled":
                    return y[r, nch]
                return y[bass.ts(r, P), bass.ts(nch, N_CHUNK)]

            def emit_tail(r, nch, ps):
                if os.environ.get("KMM_RINGS") == "tri":
                    eng = (nc.sync, nc.scalar, nc.gpsimd)[r % 3]
                else:
                    eng = nc.sync if r % 2 == 0 else nc.scalar
                if TAIL_MODE == "psum":
                    eng.dma_start(y_dst(r, nch), ps[:])
                    return
                out_sb = opool.tile([P, N_CHUNK], out_dt, tag="out_sb")
                veng = (
                    nc.vector
                    if (TAIL_ENG == "vector" or r % 2 == 0)
                    else nc.gpsimd
                )
                if TAIL_MODE == "copy":
                    veng.tensor_copy(out_sb[:], ps[:])
                else:
                    veng.tensor_add(
                        out_sb[:], ps[:], cb_sb[:, bass.ts(nch, N_CHUNK)]
                    )
                eng.dma_start(y_dst(r, nch), out_sb[:])

            def body_rgroup():
                # k-major within staggered groups of R_GROUP banks: each
                # group's tails overlap the next group's matmuls.
                if STORE_MODE == "merged":
                    big = [
                        opool.tile(
                            [P, D], out_dt, tag="big_sb", name=f"big_sb{r}"
                        )
                        for r in range(R_TILES)
                    ]
                elif STORE_MODE == "mega":
                    mega = opool.tile(
                        [P, R_TILES, D], out_dt, tag="mega_sb", name="mega_sb"
                    )
                for nch in range(N_CHUNKS):
                    groups = [
                        pspool.tile([P, N_CHUNK], mybir.dt.float32, tag="ps", name="ps")
                        for _ in range(R_TILES)
                    ]
                    for g0 in range(0, R_TILES, R_GROUP):
                        step = 0
                        for xi, mi in passes:
                            for k in range(K_TILES):
                                for r in range(g0, g0 + R_GROUP):
                                    nc.tensor.matmul(
                                        groups[r][:],
                                        x_sb[xi][:, k, bass.ts(r, P)],
                                        m_sb[mi][:, k, bass.ts(nch, N_CHUNK)],
                                        start=(step == 0),
                                        stop=(step == n_acc - 1),
                                    )
                                step += 1
                        for r in range(g0, g0 + R_GROUP):
                            if STORE_MODE == "merged":
                                nc.vector.tensor_add(
                                    big[r][:, bass.ts(nch, N_CHUNK)],
                                    groups[r][:],
                                    cb_sb[:, bass.ts(nch, N_CHUNK)],
                                )
                                if nch == N_CHUNKS - 1:
                                    eng = nc.sync if r % 2 == 0 else nc.scalar
                                    eng.dma_start(
                                        y[bass.ts(r, P), :], big[r][:]
                                    )
                            elif STORE_MODE == "mega":
                                nc.vector.tensor_add(
                                    mega[:, r, bass.ts(nch, N_CHUNK)],
                                    groups[r][:],
                                    cb_sb[:, bass.ts(nch, N_CHUNK)],
                                )
                            else:
                                emit_tail(r, nch, groups[r])
                if STORE_MODE == "mega":
                    half = R_TILES // 2
                    nc.sync.dma_start(y[:, 0:half], mega[:, 0:half])
                    nc.scalar.dma_start(y[:, half:], mega[:, half:])

            def body_kmajor():
                # k-major across 8 live PSUM banks (bank switch every MM)
                for nch in range(N_CHUNKS):
                    groups = [
                        pspool.tile([P, N_CHUNK], mybir.dt.float32, tag="ps", name="ps")
                        for _ in range(R_TILES)
                    ]
                    step = 0
                    for xi, mi in passes:
                        for k in range(K_TILES):
                            for r in range(R_TILES):
                                nc.tensor.matmul(
                                    groups[r][:],
                                    x_sb[xi][:, k, bass.ts(r, P)],
                                    m_sb[mi][:, k, bass.ts(nch, N_CHUNK)],
                                    start=(step == 0),
                                    stop=(step == n_acc - 1),
                                )
                            step += 1
                    for r in range(R_TILES):
                        emit_tail(r, nch, groups[r])

            body = body_rgroup if order == "rgroup" else body_kmajor

            if loop_iters is None:
                body()
            else:
                with tc.For_i(0, loop_iters, 1):
                    body()
    nc.compile()
    return nc


def _np_dt(mode):
    if mode in ("fp16x3", "fp16x1"):
        return np.float16
    if mode == "bf16x1":
        import ml_dtypes

        return ml_dtypes.bfloat16
    return np.float32


def _host_prep(x, Wv, bv, Wp, bp, mode=None):
    mode = mode or MM_MODE
    np_dt = _np_dt(mode)
    X = np.ascontiguousarray(x, dtype=np.float32).reshape(R_TOTAL, D)
    M64 = SCALE * (Wv.T.astype(np.float64) @ Wp.T.astype(np.float64))
    c = (SCALE * (Wp.astype(np.float64) @ bv.astype(np.float64)) + bp).astype(
        np.float32
    )
    cbt = np.ascontiguousarray(np.broadcast_to(c, (P, D)))

    if mode == "fp16x3":
        Mh = M64.astype(np.float16)
        Ml = (M64 - Mh.astype(np.float64)).astype(np.float16)
        m_arrs = {"Mh": Mh, "Ml": Ml}
    else:
        m_arrs = {"Mh": M64.astype(np_dt)}

    in_maps = []
    for i in range(N_CORES):
        shard_t = np.ascontiguousarray(X[i * R_CORE : (i + 1) * R_CORE].T)
        im = dict(m_arrs)
        im["cb"] = cbt
        if mode == "fp16x3":
            xh = shard_t.astype(np.float16)
            xl = (shard_t - xh.astype(np.float32)).astype(np.float16)
            im["xh"] = xh
            im["xl"] = xl
        else:
            im["xh"] = shard_t.astype(np_dt)
        in_maps.append(im)
    return in_maps


def kernel(x, Wq, bq, Wk, bk, Wv, bv, Wp, bp):
    x, Wv, bv, Wp, bp = (np.asarray(a) for a in (x, Wv, bv, Wp, bp))
    nc = _build_nc(MM_MODE)
    in_maps = _host_prep(x, Wv, bv, Wp, bp)
    res = run_bass_kernel_spmd(nc, in_maps, core_ids=list(range(N_CORES)))

    def fix(a):
        a = np.asarray(a)
        if STORE_MODE == "mega":  # [128, r, 1024] -> [1024, 1024]
            a = a.transpose(1, 0, 2).reshape(R_CORE, D)
        elif Y_LAYOUT == "tiled":  # [r, nch, 128, 512] -> [1024, 1024]
            a = a.transpose(0, 2, 1, 3).reshape(R_CORE, D)
        return a.astype(np.float32)

    y = np.concatenate([fix(r["y"]) for r in res.results], axis=0)
    if TAIL_MODE != "add":  # bias was not applied on device
        c = (
            SCALE * (Wp.astype(np.float64) @ bv.astype(np.float64)) + bp
        ).astype(np.float32)
        y += c
    return y.reshape(B, S, D)


# revision 37
# speedup vs baseline: 1.1016x; 1.0358x over previous
"""Trainium2 Bass kernel for nn_MultiHeadAttention_84576495993495.

Key observation: the reference module's output einsum is
    out = einsum('bhqk,bhvo->bhvo', attn, v)
which contracts softmax(attn) over BOTH q and k. Every softmax row sums
to 1, so sum_{q,k} attn == S (= 2048) and the whole attention block
collapses to out == S * v. Hence

    reference(x, ...) == ((x @ Wv.T + bv) * S) @ Wp.T + bp
                      ==  x @ M + c
with
    M = S * Wv.T @ Wp.T          (folded on host in fp64, then split)
    c = S * Wp @ bv + bp

(Verified vs the jax reference: rel Frobenius err ~3.6e-7 = fp32 noise.)

Device work: the data-dependent GEMM y = x @ M + c, sharded
data-parallel over the 8192 rows -> 1024 rows per NeuronCore.

Precision strategy: the harness tolerance is 2e-2; a SINGLE fp16 pass
(x, M both rounded to fp16, fp32 PSUM accumulate) gives rel err 2.5e-4
-- 80x inside tolerance -- at 1/3 the PE time of the fp16x3 split
scheme (which reaches 2.5e-7 but this problem does not need it).
The output is stored fp16 (adds ~2.8e-4 quantization, still ~50x
inside tolerance) which halves the output DMA traffic and doubles the
DVE tail write rate; the host upcasts to fp32.

The GEMM runs in bf16 (rel err 2.0e-3 on HW): measured ~4% faster than
fp16 at 8 concurrent cores (3.2e-4), both far inside tolerance.

Schedule (per 512-col n-chunk): k-major accumulation staggered over
two groups of 4 PSUM banks, so each group's 32 matmuls overlap the
OTHER group's tails (bias add on DVE + store). Tails alternate between
the SP and Activation HWDGE rings so output DMA never serializes
behind one descriptor queue. Microbenchmarked on HW (For_i steady
state): rgroup order with tails+DMA = 35.4us vs 43.1us for flat
k-major (1 core); pure GEMM floor is 34.7us at 8 cores (bf16 streams
at ~0.53-0.60 ns/row sustained for every operand pattern tried -- the
cost model's 2.4 GHz / 0.417 ns/row is never reached; fp8 only buys
~1.2x per row, so split-fp8 schemes lose to one bf16 pass). Groups of
2 banks, ap=256 accumulators (PSUM pool is bank-granular), fp32
output, contiguous tiled-y stores, and copy-tails-with-host-bias were
all tried on HW and did not beat this configuration.

Measured on HW (8 cores, axon, steady-state For_i slope over
T in {1, 8193, 16385} -- same protocol as the 137667ns baseline):
35.9-48.4us across repeated sessions of this exact config, rel err
2.008e-03; ~2.8-3.8x the baseline. In-session attribution at 8 cores:
GEMM only 32.3us, +DVE tails 36.3us, +stores 46.3us -- the stores are
the main non-GEMM cost and behave descriptor-rate-limited (~40M
descriptors/s/core; fp32 stores move 2x the bytes in the same time).
Merged-row stores (half the descriptors) and a whole-iteration
partition-major store (1/16th) were built and A/B'd order-controlled:
within slot-drift noise of "per" (sessions drift ~1.5us per
successive config and +/-5us session-to-session, so late-running
configs measure slow). Also tried without improvement: a third store
ring via gpsimd SWDGE, opool=32, gpsimd tails (cannot read PSUM),
DMA-from-PSUM (asserted unsupported), contiguous tiled-y stores.
"""

import os
from functools import lru_cache

import numpy as np

# Defensive: a previous run crashing mid-execution can leave the NeuronCores
# in an unrecoverable state (NRT_EXEC_UNIT_UNRECOVERABLE); resetting cores at
# NRT init clears it and is harmless otherwise.
os.environ.setdefault("NEURON_RT_RESET_CORES", "1")

import concourse.bass as bass
import concourse.mybir as mybir
import concourse.tile as tile
from concourse import bacc
from concourse.bass_utils import run_bass_kernel_spmd

N_CORES = 8
P = 128
D = 1024                       # model dim (= SLICE_SIZE)
B, S = 4, 2048
R_TOTAL = B * S                # 8192 rows
R_CORE = R_TOTAL // N_CORES    # 1024 rows per core
K_TILES = D // P               # 8
R_TILES = R_CORE // P          # 8
N_CHUNK = int(os.environ.get("KMM_NCHUNK", "512"))  # PSUM cols per group
N_CHUNKS = D // N_CHUNK
PS_BUFS = (8 * 2048) // (N_CHUNK * 4)  # PSUM accumulators that fit
R_GROUP = int(os.environ.get("KMM_RGROUP", "4"))  # banks per stagger group
SCALE = float(S)               # sum over q,k of softmax rows == S

# "bf16x1" (default) | "fp16x1" | "fp16x3" | "float32" | "float32r"
MM_MODE = os.environ.get("KMM_DTYPE", "bf16x1")
# output dtype: "f16" (default; host upcasts) | "f32"
OUT_MODE = os.environ.get("KMM_OUT", "f16")
# tail: "add" (DVE adds bias, device-complete) | "copy" (DVE copy, bias
# added on host during upcast) | "psum" (DMA directly from PSUM, bias on
# host; no DVE in the chain)
TAIL_MODE = os.environ.get("KMM_TAIL", "add")
# y layout: "rowmajor" | "tiled" ([r, nch, 128, 512] so each store is one
# fully contiguous DRAM block; host reassembles)
Y_LAYOUT = os.environ.get("KMM_YLAYOUT", "rowmajor")
OPOOL_BUFS = int(os.environ.get("KMM_OPOOL", "16"))
# tail engine: "vector" | "split" (alternate DVE / GpSimd)
TAIL_ENG = os.environ.get("KMM_TAILENG", "vector")
# store granularity: "per" (one store per bank+nchunk, 1KB DRAM runs) |
# "merged" (both nchunks collected in one [P, D] tile per bank, stored as
# full 2KB rows -> half the DMA descriptors) | "mega" (whole iteration in
# one [P, R_TILES, D] tile, y kept partition-major in DRAM and reassembled
# on host -> 8KB runs, ~128 descriptors/iter; stores are descriptor-rate
# limited at 8 cores) | "rowgroup" (row-major iteration order: finish both
# n-chunks of a 4-bank group before the next group, so full 2KB rows ship
# early AND spread across the iteration)
STORE_MODE = os.environ.get("KMM_STORE", "per")


def _mm_dt(mode):
    if mode in ("fp16x3", "fp16x1"):
        return mybir.dt.float16
    if mode == "bf16x1":
        return mybir.dt.bfloat16
    return getattr(mybir.dt, mode)


@lru_cache(maxsize=4)
def _build_nc(mode: str, loop_iters: int | None = None, order: str | None = None):
    """loop_iters: when set, wrap the compute body in a tc.For_i hardware
    loop (inputs loaded once) -- used by the benchmark harness to measure
    steady-state per-iteration device time without NTFF profiling."""
    if order is None:
        order = os.environ.get("KMM_ORDER", "rgroup")
    split = mode == "fp16x3"
    mm_dt = _mm_dt(mode)
    out_dt = mybir.dt.float16 if OUT_MODE == "f16" else mybir.dt.float32
    if TAIL_MODE == "psum":
        out_dt = mybir.dt.float32  # DMA cannot convert dtypes on HWDGE
    nc = bacc.Bacc(None, target_bir_lowering=False)

    if split:
        x_names, m_names = ["xh", "xl"], ["Mh", "Ml"]
    else:
        x_names, m_names = ["xh"], ["Mh"]
    x_dram = [
        nc.dram_tensor(n, [D, R_CORE], mm_dt, kind="ExternalInput") for n in x_names
    ]
    m_dram = [nc.dram_tensor(n, [D, D], mm_dt, kind="ExternalInput") for n in m_names]
    cb = nc.dram_tensor("cb", [P, D], mybir.dt.float32, kind="ExternalInput")
    if STORE_MODE == "mega":
        y = nc.dram_tensor(
            "y", [P, R_TILES, D], out_dt, kind="ExternalOutput"
        )
    elif Y_LAYOUT == "tiled":
        y = nc.dram_tensor(
            "y", [R_TILES, N_CHUNKS, P, N_CHUNK], out_dt, kind="ExternalOutput"
        )
    else:
        y = nc.dram_tensor("y", [R_CORE, D], out_dt, kind="ExternalOutput")

    x_t = [t.rearrange("(ko p) r -> p ko r", p=P) for t in x_dram]   # [128, 8, 1024]
    m_t = [t.rearrange("(ko p) n -> p ko n", p=P) for t in m_dram]   # [128, 8, 1024]

    # (x operand, M operand) per accumulation pass; the xl@Ml term is dropped.
    passes = [(0, 0), (0, 1), (1, 0)] if split else [(0, 0)]

    with tile.TileContext(nc) as tc:
        with (
            tc.tile_pool(name="wpool", bufs=1) as wpool,
            tc.tile_pool(
                name="opool",
                bufs=2 if STORE_MODE == "mega" else OPOOL_BUFS,
            ) as opool,
            tc.tile_pool(name="pspool", bufs=PS_BUFS, space="PSUM") as pspool,
        ):
            x_sb = [
                wpool.tile([P, K_TILES, R_CORE], mm_dt, tag=f"x_sb{i}", name=f"x_sb{i}")
                for i in range(len(x_dram))
            ]
            m_sb = [
                wpool.tile([P, K_TILES, D], mm_dt, tag=f"m_sb{i}", name=f"m_sb{i}")
                for i in range(len(m_dram))
            ]
            cb_sb = wpool.tile([P, D], mybir.dt.float32, tag="cb_sb")

            if TAIL_MODE == "add":
                nc.sync.dma_start(cb_sb[:], cb[:])
            # Load in consumption order, alternating the two HWDGE rings.
            for i in range(len(x_dram)):
                for k in range(K_TILES):
                    nc.sync.dma_start(x_sb[i][:, k], x_t[i][:, k])
                    for nch in range(N_CHUNKS):
                        nc.scalar.dma_start(
                            m_sb[i][:, k, bass.ts(nch, N_CHUNK)],
                            m_t[i][:, k, bass.ts(nch, N_CHUNK)],
                        )

            n_acc = len(passes) * K_TILES

            def y_dst(r, nch):
                if Y_LAYOUT == "tiled":
                    return y[r, nch]
                return y[bass.ts(r, P), bass.ts(nch, N_CHUNK)]

            def emit_tail(r, nch, ps):
                if os.environ.get("KMM_RINGS") == "tri":
                    eng = (nc.sync, nc.scalar, nc.gpsimd)[r % 3]
                else:
                    eng = nc.sync if r % 2 == 0 else nc.scalar
                if TAIL_MODE == "psum":
                    eng.dma_start(y_dst(r, nch), ps[:])
                    return
                out_sb = opool.tile([P, N_CHUNK], out_dt, tag="out_sb")
                veng = (
                    nc.vector
                    if (TAIL_ENG == "vector" or r % 2 == 0)
                    else nc.gpsimd
                )
                if TAIL_MODE == "copy":
                    veng.tensor_copy(out_sb[:], ps[:])
                else:
                    veng.tensor_add(
                        out_sb[:], ps[:], cb_sb[:, bass.ts(nch, N_CHUNK)]
                    )
                eng.dma_start(y_dst(r, nch), out_sb[:])

            def body_rgroup():
                # k-major within staggered groups of R_GROUP banks: each
                # group's tails overlap the next group's matmuls.
                if STORE_MODE == "merged":
                    big = [
                        opool.tile(
                            [P, D], out_dt, tag="big_sb", name=f"big_sb{r}"
                        )
                        for r in range(R_TILES)
                    ]
                elif STORE_MODE == "mega":
                    mega = opool.tile(
                        [P, R_TILES, D], out_dt, tag="mega_sb", name="mega_sb"
                    )
                for nch in range(N_CHUNKS):
                    groups = [
                        pspool.tile([P, N_CHUNK], mybir.dt.float32, tag="ps", name="ps")
                        for _ in range(R_TILES)
                    ]
                    for g0 in range(0, R_TILES, R_GROUP):
                        step = 0
                        for xi, mi in passes:
                            for k in range(K_TILES):
                                for r in range(g0, g0 + R_GROUP):
                                    nc.tensor.matmul(
                                        groups[r][:],
                                        x_sb[xi][:, k, bass.ts(r, P)],
                                        m_sb[mi][:, k, bass.ts(nch, N_CHUNK)],
                                        start=(step == 0),
                                        stop=(step == n_acc - 1),
                                    )
                                step += 1
                        for r in range(g0, g0 + R_GROUP):
                            if STORE_MODE == "merged":
                                nc.vector.tensor_add(
                                    big[r][:, bass.ts(nch, N_CHUNK)],
                                    groups[r][:],
                                    cb_sb[:, bass.ts(nch, N_CHUNK)],
                                )
                                if nch == N_CHUNKS - 1:
                                    eng = nc.sync if r % 2 == 0 else nc.scalar
                                    eng.dma_start(
                                        y[bass.ts(r, P), :], big[r][:]
                                    )
                            elif STORE_MODE == "mega":
                                nc.vector.tensor_add(
                                    mega[:, r, bass.ts(nch, N_CHUNK)],
                                    groups[r][:],
                                    cb_sb[:, bass.ts(nch, N_CHUNK)],
                                )
                            else:
                                emit_tail(r, nch, groups[r])
                if STORE_MODE == "mega":
                    half = R_TILES // 2
                    nc.sync.dma_start(y[:, 0:half], mega[:, 0:half])
                    nc.scalar.dma_start(y[:, half:], mega[:, half:])

            def body_kmajor():
                # k-major across 8 live PSUM banks (bank switch every MM)
                for nch in range(N_CHUNKS):
                    groups = [
                        pspool.tile([P, N_CHUNK], mybir.dt.float32, tag="ps", name="ps")
                        for _ in range(R_TILES)
                    ]
                    step = 0
                    for xi, mi in passes:
                        for k in range(K_TILES):
                            for r in range(R_TILES):
                                nc.tensor.matmul(
                                    groups[r][:],
                                    x_sb[xi][:, k, bass.ts(r, P)],
                                    m_sb[mi][:, k, bass.ts(nch, N_CHUNK)],
                                    start=(step == 0),
                                    stop=(step == n_acc - 1),
                                )
                            step += 1
                    for r in range(R_TILES):
                        emit_tail(r, nch, groups[r])

            def body_rowgroup():
                # Row-major: complete BOTH n-chunks of each 4-bank group,
                # then store that group's full 2KB output rows while the
                # next group computes. Halves store descriptors vs "per"
                # without bunching them at the iteration end.
                big = [
                    opool.tile([P, D], out_dt, tag="big_sb", name=f"big_sb{r}")
                    for r in range(R_TILES)
                ]
                for g0 in range(0, R_TILES, R_GROUP):
                    for nch in range(N_CHUNKS):
                        ps = {
                            r: pspool.tile(
                                [P, N_CHUNK], mybir.dt.float32,
                                tag="ps", name="ps",
                            )
                            for r in range(g0, g0 + R_GROUP)
                        }
                        step = 0
                        for xi, mi in passes:
                            for k in range(K_TILES):
                                for r in range(g0, g0 + R_GROUP):
                                    nc.tensor.matmul(
                                        ps[r][:],
                                        x_sb[xi][:, k, bass.ts(r, P)],
                                        m_sb[mi][:, k, bass.ts(nch, N_CHUNK)],
                                        start=(step == 0),
                                        stop=(step == n_acc - 1),
                                    )
                                step += 1
                        for r in range(g0, g0 + R_GROUP):
                            nc.vector.tensor_add(
                                big[r][:, bass.ts(nch, N_CHUNK)],
                                ps[r][:],
                                cb_sb[:, bass.ts(nch, N_CHUNK)],
                            )
                    for r in range(g0, g0 + R_GROUP):
                        eng = nc.sync if r % 2 == 0 else nc.scalar
                        eng.dma_start(y[bass.ts(r, P), :], big[r][:])

            if STORE_MODE == "rowgroup":
                body = body_rowgroup
            else:
                body = body_rgroup if order == "rgroup" else body_kmajor

            if loop_iters is None:
                body()
            else:
                with tc.For_i(0, loop_iters, 1):
                    body()
    nc.compile()
    return nc


def _np_dt(mode):
    if mode in ("fp16x3", "fp16x1"):
        return np.float16
    if mode == "bf16x1":
        import ml_dtypes

        return ml_dtypes.bfloat16
    return np.float32


def _host_prep(x, Wv, bv, Wp, bp, mode=None):
    mode = mode or MM_MODE
    np_dt = _np_dt(mode)
    X = np.ascontiguousarray(x, dtype=np.float32).reshape(R_TOTAL, D)
    M64 = SCALE * (Wv.T.astype(np.float64) @ Wp.T.astype(np.float64))
    c = (SCALE * (Wp.astype(np.float64) @ bv.astype(np.float64)) + bp).astype(
        np.float32
    )
    cbt = np.ascontiguousarray(np.broadcast_to(c, (P, D)))

    if mode == "fp16x3":
        Mh = M64.astype(np.float16)
        Ml = (M64 - Mh.astype(np.float64)).astype(np.float16)
        m_arrs = {"Mh": Mh, "Ml": Ml}
    else:
        m_arrs = {"Mh": M64.astype(np_dt)}

    in_maps = []
    for i in range(N_CORES):
        shard_t = np.ascontiguousarray(X[i * R_CORE : (i + 1) * R_CORE].T)
        im = dict(m_arrs)
        im["cb"] = cbt
        if mode == "fp16x3":
            xh = shard_t.astype(np.float16)
            xl = (shard_t - xh.astype(np.float32)).astype(np.float16)
            im["xh"] = xh
            im["xl"] = xl
        else:
            im["xh"] = shard_t.astype(np_dt)
        in_maps.append(im)
    return in_maps


def kernel(x, Wq, bq, Wk, bk, Wv, bv, Wp, bp):
    x, Wv, bv, Wp, bp = (np.asarray(a) for a in (x, Wv, bv, Wp, bp))
    nc = _build_nc(MM_MODE)
    in_maps = _host_prep(x, Wv, bv, Wp, bp)
    res = run_bass_kernel_spmd(nc, in_maps, core_ids=list(range(N_CORES)))

    def fix(a):
        a = np.asarray(a)
        if STORE_MODE == "mega":  # [128, r, 1024] -> [1024, 1024]
            a = a.transpose(1, 0, 2).reshape(R_CORE, D)
        elif Y_LAYOUT == "tiled":  # [r, nch, 128, 512] -> [1024, 1024]
            a = a.transpose(0, 2, 1, 3).reshape(R_CORE, D)
        return a.astype(np.float32)

    y = np.concatenate([fix(r["y"]) for r in res.results], axis=0)
    if TAIL_MODE != "add":  # bias was not applied on device
        c = (
            SCALE * (Wp.astype(np.float64) @ bv.astype(np.float64)) + bp
        ).astype(np.float32)
        y += c
    return y.reshape(B, S, D)
